# revision 2
# baseline (speedup 1.0000x reference)
"""AnisoMultiGaussSpatialConv on 8 TRN2 NeuronCores.

Math: out[b,n,f] = sum_m K[b,n,m] * y_fea[b,m,f]
      K = sum_k w_k exp(-a_k * d),  d = (x_n-y_m)^T Gamma_m (x_n-y_m),
      a = (200, 50, 12.5), w = (0.2, 0.3, 0.5).

Decomposition per core (N sharded over 8 cores, 512 target rows each):
  d^T[m,n] = sum_p G_ext[p,m] * X_ext[p,n]  (K=13 -> bf16 hi/lo split, K=52)
  u = exp(-12.5 d + ln .5)           (ACT)         -> w3 term
  v = exp(-200  d + ln .2)           (ACT)         -> w1 term
  s2 = (u*u)^2 = .5^4 exp(-50 d)     (DVE)         -> w2 term via scaled y_fea
  outT[f,n] += yfea^T (u+v)  +  (4.8*yfea)^T s2    (PE, PSUM accumulation)
Host transposes outT back to [n,f].
"""

import copy
import math

import numpy as np
import ml_dtypes

B, N, M, D, F = 2, 4096, 4096, 3, 64
NCORES = 8
NLOC = N // NCORES          # 512 target rows per core
KSTACK = 52                 # 13 ext components x (hi,lo)x(hi,lo) pairing
GSZ = 3                     # m-tiles (of 128 rows) per processing group
NMT = M // 128              # 32 m-tiles
A1, A2, A3 = 200.0, 50.0, 12.5
W1, W2, W3 = 0.2, 0.3, 0.5
C2 = W2 / W3 ** 4           # scale for the s2 (sigma=0.1) term

_BF16 = ml_dtypes.bfloat16

_cache = {}


def _split_multiwaits(nc, mybir, bass, max_waits=1):
    """This walrus build caps sync-wait commands per instruction; hoist
    extra waits onto single-wait NOPs preceding the instruction on the
    same engine (sequencers execute in order, so semantics unchanged)."""
    scratch = bass.Bass()
    tpl = scratch.vector.nop(hint="sw").ins
    ctr = 0
    for fn in nc.m.functions:
        for bb in fn.blocks:
            out = []
            changed = False
            for inst in bb.instructions:
                si = inst.sync_info
                ow = list(si.on_wait) if si is not None and si.on_wait else []
                if len(ow) > max_waits:
                    changed = True
                    extra, keep = ow[:-max_waits], ow[-max_waits:]
                    for w in extra:
                        nop = copy.deepcopy(tpl)
                        nop.name = f"SWN-{ctr}"
                        ctr += 1
                        nop.engine = inst.engine
                        nop.sync_info = mybir.SyncInfo(on_wait=[w], on_update=[])
                        out.append(nop)
                    si.on_wait = keep
                    inst.sync_info = si
                out.append(inst)
            if changed:
                bb.instructions = out
    return ctr


def _build():
    if "nc" in _cache:
        return _cache["nc"]
    import concourse.bass as bass
    import concourse.mybir as mybir
    from concourse.tile import TileContext

    fp32 = mybir.dt.float32
    bf16 = mybir.dt.bfloat16
    EXP = mybir.ActivationFunctionType.Exp

    nc = bass.Bass()
    # activation() biases must be pre-registered const APs
    for val in (math.log(W3), math.log(W1)):
        t = nc.alloc_sbuf_tensor(f"const-f32-{val}", [128, 1], fp32)
        nc.gpsimd.memset(t.ap(), val)
        nc.const_aps.aps[(fp32, val)] = t.ap()
    nc.all_engine_barrier()

    gstack_d = nc.declare_dram_parameter("gstack", [B, KSTACK, M], bf16, isOutput=False)
    xstack_d = nc.declare_dram_parameter("xstack", [B, KSTACK, NLOC], bf16, isOutput=False)
    yf_d = nc.declare_dram_parameter("yf", [B, 128, NMT, F], bf16, isOutput=False)
    yf2_d = nc.declare_dram_parameter("yf2", [B, 128, NMT, F], bf16, isOutput=False)
    out_d = nc.declare_dram_parameter("out", [B, F, NLOC], fp32, isOutput=True)

    groups = []
    t0 = 0
    while t0 < NMT:
        groups.append((t0, min(GSZ, NMT - t0)))
        t0 += GSZ

    with TileContext(nc) as tc:
        with (
            tc.tile_pool(name="persist", bufs=1) as persist,
            tc.tile_pool(name="work", bufs=3) as work,
            tc.tile_pool(name="osb", bufs=2) as osb,
            tc.tile_pool(name="dpsum", bufs=2, space="PSUM") as dpool,
            tc.tile_pool(name="opsum", bufs=2, space="PSUM") as opool,
        ):
            gstack = []
            xstack = []
            yf = []
            yf2 = []
            for b in range(B):
                g = persist.tile([KSTACK, M], bf16, tag=f"g{b}")
                nc.sync.dma_start(out=g[:], in_=gstack_d[b])
                gstack.append(g)
                xs = persist.tile([KSTACK, NLOC], bf16, tag=f"x{b}")
                nc.sync.dma_start(out=xs[:], in_=xstack_d[b])
                xstack.append(xs)
                t = persist.tile([128, NMT, F], bf16, tag=f"yf{b}")
                nc.sync.dma_start(out=t[:], in_=yf_d[b])
                yf.append(t)
                t2 = persist.tile([128, NMT, F], bf16, tag=f"yf2{b}")
                nc.sync.dma_start(out=t2[:], in_=yf2_d[b])
                yf2.append(t2)

            for b in range(B):
                oacc = opool.tile([F, NLOC], fp32, tag="oacc")
                for gi, (t0, gsz) in enumerate(groups):
                    dp = dpool.tile([128, GSZ * NLOC], fp32, tag="dp")
                    for j in range(gsz):
                        mt = t0 + j
                        nc.tensor.matmul(
                            dp[:, j * NLOC:(j + 1) * NLOC],
                            lhsT=gstack[b][:, mt * 128:(mt + 1) * 128],
                            rhs=xstack[b][:],
                            start=True,
                            stop=True,
                        )
                    fd = gsz * NLOC
                    u = work.tile([128, GSZ * NLOC], bf16, tag="u")
                    nc.scalar.activation(u[:, :fd], dp[:, :fd], EXP,
                                         bias=math.log(W3), scale=-A3)
                    v = work.tile([128, GSZ * NLOC], bf16, tag="v")
                    nc.scalar.activation(v[:, :fd], dp[:, :fd], EXP,
                                         bias=math.log(W1), scale=-A1)
                    w = work.tile([128, GSZ * NLOC], bf16, tag="w")
                    nc.vector.tensor_add(w[:, :fd], u[:, :fd], v[:, :fd])
                    s = work.tile([128, GSZ * NLOC], bf16, tag="s")
                    nc.vector.tensor_mul(s[:, :fd], u[:, :fd], u[:, :fd])
                    s2 = work.tile([128, GSZ * NLOC], bf16, tag="s2")
                    nc.vector.tensor_mul(s2[:, :fd], s[:, :fd], s[:, :fd])
                    for j in range(gsz):
                        mt = t0 + j
                        nc.tensor.matmul(
                            oacc[:],
                            lhsT=yf[b][:, mt, :],
                            rhs=w[:, j * NLOC:(j + 1) * NLOC],
                            start=(mt == 0),
                            stop=False,
                        )
                        nc.tensor.matmul(
                            oacc[:],
                            lhsT=yf2[b][:, mt, :],
                            rhs=s2[:, j * NLOC:(j + 1) * NLOC],
                            start=False,
                            stop=(mt == NMT - 1),
                        )
                ot = osb.tile([F, NLOC], fp32, tag="ot")
                nc.vector.tensor_copy(ot[:], oacc[:])
                nc.sync.dma_start(out=out_d[b], in_=ot[:])

    _split_multiwaits(nc, mybir, bass)
    _cache["nc"] = nc
    return nc


def _bf_split(v):
    hi = v.astype(_BF16).astype(np.float32)
    lo = (v - hi).astype(_BF16)
    return hi.astype(_BF16), lo


def _prep(x, y, y_fea, gamma):
    x = np.ascontiguousarray(x, np.float32)
    y = np.ascontiguousarray(y, np.float32)
    y_fea = np.ascontiguousarray(y_fea, np.float32)
    gamma = np.ascontiguousarray(gamma, np.float32)

    X2 = (x[:, :, :, None] * x[:, :, None, :]).reshape(B, N, 9)
    Gflat = gamma.reshape(B, M, 9)
    Gy = np.einsum("bmde,bme->bmd", gamma, y)
    yGy = np.einsum("bmd,bmd->bm", y, Gy)
    G_ext = np.concatenate([Gflat, -2.0 * Gy, yGy[:, :, None]], axis=2)
    X_ext = np.concatenate([X2, x, np.ones((B, N, 1), np.float32)], axis=2)

    Ghi, Glo = _bf_split(G_ext)
    Xhi, Xlo = _bf_split(X_ext)
    # sum_p X*G = Xhi*Ghi + Xhi*Glo + Xlo*Ghi + Xlo*Glo
    Gs = np.concatenate([Ghi, Glo, Ghi, Glo], axis=2)  # [B,M,52]
    Xs = np.concatenate([Xhi, Xhi, Xlo, Xlo], axis=2)  # [B,N,52]
    gstack = np.ascontiguousarray(Gs.transpose(0, 2, 1))  # [B,52,M]
    xstackT = np.ascontiguousarray(Xs.transpose(0, 2, 1))  # [B,52,N]

    # [B, M, F] -> [B, 128, NMT, F] partition-major for one contiguous DMA
    yf = np.ascontiguousarray(
        y_fea.reshape(B, NMT, 128, F).transpose(0, 2, 1, 3)).astype(_BF16)
    yf2 = np.ascontiguousarray(
        (C2 * y_fea).reshape(B, NMT, 128, F).transpose(0, 2, 1, 3)).astype(_BF16)
    return gstack, xstackT, yf, yf2


def kernel(x, y, y_fea, gamma):
    from concourse.bass_utils import run_bass_kernel_spmd

    assert x.shape == (B, N, D) and y.shape == (B, M, D)
    assert y_fea.shape == (B, M, F) and gamma.shape == (B, M, D, D)

    gstack, xstackT, yf, yf2 = _prep(x, y, y_fea, gamma)
    in_maps = []
    for c in range(NCORES):
        in_maps.append({
            "gstack": gstack,
            "xstack": np.ascontiguousarray(
                xstackT[:, :, c * NLOC:(c + 1) * NLOC]),
            "yf": yf,
            "yf2": yf2,
        })

    nc = _build()
    res = run_bass_kernel_spmd(nc, in_maps, core_ids=list(range(NCORES)))

    out = np.empty((B, N, F), np.float32)
    for c in range(NCORES):
        o = res.results[c]["out"]  # [B, F, NLOC]
        out[:, c * NLOC:(c + 1) * NLOC, :] = o.transpose(0, 2, 1)
    return out
